# revision 27
# baseline (speedup 1.0000x reference)
"""CostVolumeLayer Trainium2 kernel.

Problem: src, tgt [B=8, C=128, H=160, W=288] fp32.
out[b, k, y, x] = (1/C) * sum_c src[b,c,y,x] * tgt[b,c,y+dy_k,x+dx_k]
for the 81 displacements (dy,dx) in [-4,4]^2 (torch CostVolume channel order),
with zero padding outside the image.

Strategy (data-parallel over batch, one batch per NeuronCore):
  - For each 8x16 tile of src positions, the PE computes the Gram block
    src_tile[C, 128].T @ tgt_window[C, 16x24] -> PSUM [128 pos, 384 window].
    Every (pos, k) output is some element of this block (banded diagonals).
    8x16 tiles minimize window area per position (384 cols vs 480 for 4x32)
    - the Gram write to HBM is the kernel's bottleneck (35.4 MB/core).
  - A naive 16-matmul-per-tile schedule (one 24-col matmul per window row)
    is PE-bound on weight reloads. Instead the window rows are pre-packed:
    for every 8-row group of padded tgt, a DVE/ACT copy builds
    pack_g[C, xtile(18) x row(8) x col(24)] whose per-tile 192-element runs
    are contiguous, so each position tile needs only TWO 192-col matmuls
    (lower / upper half of its 16-row window). Each tgt row is packed once
    per group but used by two strips.
  - PSUM tiles hold two position tiles (2 banks, 384-col blocks at 512-col
    offsets); DVE/ACT alternate evacuating pairs (x 1/C, cast bf16),
    halving per-instruction overhead. Raw Gram blocks are DMA'd to DRAM
    densely, spread over all three DMA rings (SWDGE + SP + ACT HWDGE).
  - The host (this file, numpy) de-shears the banded diagonals into the
    [B, 81, H, W] output. The per-position diagonal gather needs a
    partition-coupled byte offset which neither HWDGE nor SWDGE descriptor
    generation supports (verified on HW: HWDGE wraps the sub-row offset
    mod 4 elements, SWDGE emits garbage), so it stays on the host where it
    is a cheap vectorized gather.
  - Inputs are cast to bf16 on the host (halves HBM read traffic); PSUM
    accumulation is fp32. tgt is zero-padded by S=4 on the host so all
    device DMAs are fully contiguous per partition.
"""

import sys

for _p in ("/opt/trn_rl_repo",):
    if _p not in sys.path:
        sys.path.insert(0, _p)

import numpy as np
import ml_dtypes

import concourse.mybir as mybir
import concourse.tile as tile
from concourse import bacc
from concourse.ap import AP
from concourse.bass_utils import run_bass_kernel_spmd

B, C, H, W, S = 8, 128, 160, 288, 4
TY, TX = 8, 16                      # src tile = 8x16 = 128 positions (PSUM partitions)
WIN_Y, WIN_X = TY + 2 * S, TX + 2 * S   # 16 x 24 tgt window
NWIN = WIN_Y * WIN_X                # 384 PSUM columns per tile
NSTRIP = H // TY                    # 20 row strips
NXT = W // TX                       # 18 x tiles per strip
HP, WP = H + 2 * S, W + 2 * S       # padded tgt dims (168, 296)
TGT_CHUNK = 24                      # tgt rows per chunk tile (168 = 7 x 24; never
                                    # splits an 8-row pack group)
N_TGT_CHUNKS = HP // TGT_CHUNK      # 7
N_PACK = HP // TY                   # 21 8-row pack groups
PACK_COLS = NXT * TY * WIN_X        # 3456 packed elements per partition
HALF = TY * WIN_X                   # 192 columns per half-window matmul
PSUM_PAIR = 1024                    # 2 banks: tile j of pair at cols 512*j
N_CORES = 8

BF16 = mybir.dt.bfloat16
NP_BF16 = ml_dtypes.bfloat16


def _displacements(s):
    d = [(0, 0)]
    for i in range(1, s + 1):
        d += [(-i, 0), (i, 0), (0, -i), (0, i)]
        for j in range(1, s + 1):
            d += [(-i, -j), (i, j), (-i, j), (i, -j)]
    return d


DISPLACEMENTS = _displacements(S)


def _build_bass():
    nc = bacc.Bacc(
        "TRN2",
        target_bir_lowering=False,
        debug=False,
        num_devices=N_CORES,
    )
    # src pre-tiled on host: [C, NSTRIP, NXT, TY*TX] so each tile's lhsT is
    # one contiguous 128-element slice.
    src_t = nc.dram_tensor(
        "src", [C, NSTRIP, NXT, TY * TX], BF16, kind="ExternalInput"
    ).ap()
    tgt_t = nc.dram_tensor("tgtp", [C, HP, WP], BF16, kind="ExternalInput").ap()
    out_t = nc.dram_tensor(
        "gram", [NSTRIP, C, NXT, NWIN], BF16, kind="ExternalOutput"
    ).ap()

    with tile.TileContext(nc) as tc:
        with (
            tc.tile_pool(name="tgtres", bufs=1) as tgt_pool,
            tc.tile_pool(name="pack", bufs=4) as pack_pool,
            tc.tile_pool(name="srcstrip", bufs=5) as src_pool,
            tc.tile_pool(name="outstrip", bufs=3) as out_pool,
            tc.tile_pool(name="psum", bufs=4, space="PSUM") as psum_pool,
        ):
            # tgt resident in SBUF, loaded up front in row chunks (separate
            # tiles, ACT HWDGE ring) so packs only depend on their chunk.
            # Rescheduling this preload (just-in-time chunk loads, delaying
            # a ring's write share past it, or moving the loads to SWDGE)
            # all measured neutral-to-slower - the plain burst overlaps
            # best.
            tgt_chunks = []
            for ci in range(N_TGT_CHUNKS):
                ch = tgt_pool.tile([C, TGT_CHUNK * WP], BF16, tag=f"tgtc{ci}")
                nc.scalar.dma_start(
                    ch[:], tgt_t[:, ci * TGT_CHUNK : (ci + 1) * TGT_CHUNK, :]
                )
                tgt_chunks.append(ch)

            packs = {}

            def ensure_pack(g):
                # pack_g[p, t*192 + row*24 + col] = tgt[p, 8g + row, 16t + col]
                if g not in packs:
                    r0 = TY * g
                    ch = tgt_chunks[r0 // TGT_CHUNK]
                    cap = ch[:]
                    src_ap = AP(
                        cap.tensor,
                        cap.offset + (r0 % TGT_CHUNK) * WP,
                        [[cap.shape[1], C], [TX, NXT], [WP, TY], [1, WIN_X]],
                    )
                    pk = pack_pool.tile([C, PACK_COLS], BF16)
                    dst = pk[:].rearrange("p (t r c) -> p t r c", r=TY, c=WIN_X)
                    # DVE/ACT alternate the pack copies (GPSIMD's Q7 cores
                    # are ~4x slower per element and serialize with SWDGE
                    # descriptor generation - measured 267us busy).
                    if g % 2 == 0:
                        nc.vector.tensor_scalar_mul(dst, src_ap, 1.0)
                    else:
                        nc.scalar.mul(dst, src_ap, 1.0)
                    packs[g] = pk
                return packs[g]

            for s in range(NSTRIP):
                src_tile = src_pool.tile([C, NXT * TY * TX], BF16)
                nc.sync.dma_start(src_tile[:], src_t[:, s])
                src_view = src_tile.rearrange("p (t m) -> p t m", m=TY * TX)

                pk_lo = ensure_pack(s)
                pk_hi = ensure_pack(s + 1)

                out_tile = out_pool.tile([C, NXT * NWIN], BF16)
                out_view = out_tile.rearrange("p (t w) -> p t w", w=NWIN)

                for pi in range(NXT // 2):
                    t0 = 2 * pi
                    ps = psum_pool.tile([C, PSUM_PAIR], mybir.dt.float32)
                    for j in (0, 1):
                        t = t0 + j
                        base = 512 * j
                        nc.tensor.matmul(
                            ps[:, base : base + HALF],
                            lhsT=src_view[:, t, :],
                            rhs=pk_lo[:, t * HALF : (t + 1) * HALF],
                            start=True,
                            stop=True,
                        )
                        nc.tensor.matmul(
                            ps[:, base + HALF : base + NWIN],
                            lhsT=src_view[:, t, :],
                            rhs=pk_hi[:, t * HALF : (t + 1) * HALF],
                            start=True,
                            stop=True,
                        )
                    # Evacuate both tiles of the pair in one instruction.
                    pv = ps.rearrange("p (j c) -> p j c", c=512)[:, :, :NWIN]
                    dst = out_view[:, t0 : t0 + 2, :]
                    if pi % 2 == 0:
                        nc.vector.tensor_scalar_mul(dst, pv, 1.0 / C)
                    else:
                        nc.scalar.mul(dst, pv, 1.0 / C)
                    if s == NSTRIP - 1:
                        # Last strip: write each pair as soon as it is
                        # evacuated (HWDGE rings) so the final drain is one
                        # pair, not a whole third of the strip.
                        eng = nc.sync if pi % 2 == 0 else nc.scalar
                        eng.dma_start(
                            out_t[s, :, t0 : t0 + 2],
                            out_tile[:, t0 * NWIN : (t0 + 2) * NWIN],
                        )

                if s < NSTRIP - 1:
                    # Spread the gram writes (the dominant HBM traffic)
                    # across all three DMA rings so no single descriptor
                    # queue serializes them.
                    third = (NXT // 3) * NWIN
                    nc.gpsimd.dma_start(
                        out_t[s, :, : NXT // 3], out_tile[:, :third]
                    )
                    nc.sync.dma_start(
                        out_t[s, :, NXT // 3 : 2 * (NXT // 3)],
                        out_tile[:, third : 2 * third],
                    )
                    nc.scalar.dma_start(
                        out_t[s, :, 2 * (NXT // 3) :], out_tile[:, 2 * third :]
                    )

    nc.compile()
    return nc


_NC = None


def _get_nc():
    global _NC
    if _NC is None:
        _NC = _build_bass()
    return _NC


def _run_device(src_bf, tgtp_bf, **run_kwargs):
    nc = _get_nc()
    in_maps = [{"src": src_bf[b], "tgtp": tgtp_bf[b]} for b in range(B)]
    return run_bass_kernel_spmd(nc, in_maps, core_ids=list(range(N_CORES)), **run_kwargs)


def _deshear(gram):
    """gram: [B, NSTRIP, C, NXT, NWIN] (any float dtype) -> [B, 81, H, W] fp32."""
    g = np.asarray(gram, dtype=np.float32).reshape(
        B, NSTRIP, TY, TX, NXT, WIN_Y, WIN_X
    )
    out = np.empty((B, len(DISPLACEMENTS), H, W), np.float32)
    yy = np.arange(TY)[:, None]
    xx = np.arange(TX)[None, :]
    for k, (dy, dx) in enumerate(DISPLACEMENTS):
        # v axes: (yy, xx, b, strip, xtile)
        v = g[:, :, yy, xx, :, yy + dy + S, xx + dx + S]
        out[:, k] = v.transpose(2, 3, 0, 4, 1).reshape(B, H, W)
    return out


def kernel(src, tgt, _profile_out=None):
    src = np.asarray(src)
    tgt = np.asarray(tgt)
    assert src.shape == (B, C, H, W) and tgt.shape == (B, C, H, W)

    # [B, C, H, W] -> [B, C, NSTRIP, TY, NXT, TX] -> [B, C, NSTRIP, NXT, TY*TX]
    src_bf = np.ascontiguousarray(
        src.astype(NP_BF16)
        .reshape(B, C, NSTRIP, TY, NXT, TX)
        .transpose(0, 1, 2, 4, 3, 5)
        .reshape(B, C, NSTRIP, NXT, TY * TX)
    )
    tgtp_bf = np.zeros((B, C, HP, WP), NP_BF16)
    tgtp_bf[:, :, S : S + H, S : S + W] = tgt.astype(NP_BF16)

    kw = {}
    if _profile_out is not None:
        kw["trace"] = True
        import os, shutil

        td = "/tmp/bass_trace"
        shutil.rmtree(td, ignore_errors=True)
        os.makedirs(td, exist_ok=True)
        kw["tmpdir"] = td
    res = _run_device(src_bf, tgtp_bf, **kw)
    if _profile_out is not None:
        _profile_out.update(
            exec_time_ns=res.exec_time_ns,
            mean_exec_time_ns=res.mean_exec_time_ns,
        )

    gram = np.stack([res.results[b]["gram"] for b in range(B)])
    return _deshear(gram)


# revision 31
# speedup vs baseline: 1.0392x; 1.0392x over previous
"""CostVolumeLayer Trainium2 kernel.

Problem: src, tgt [B=8, C=128, H=160, W=288] fp32.
out[b, k, y, x] = (1/C) * sum_c src[b,c,y,x] * tgt[b,c,y+dy_k,x+dx_k]
for the 81 displacements (dy,dx) in [-4,4]^2 (torch CostVolume channel order),
with zero padding outside the image.

Strategy (data-parallel over batch, one batch per NeuronCore):
  - For each 8x16 tile of src positions, the PE computes the Gram block
    src_tile[C, 128].T @ tgt_window[C, 16x24] -> PSUM [128 pos, 384 window].
    Every (pos, k) output is some element of this block (banded diagonals).
    8x16 tiles minimize window area per position (384 cols vs 480 for 4x32)
    - the Gram write to HBM is the kernel's bottleneck (35.4 MB/core).
  - A naive 16-matmul-per-tile schedule (one 24-col matmul per window row)
    is PE-bound on weight reloads. Instead the window rows are pre-packed:
    for every 8-row group of padded tgt, a DVE/ACT copy builds
    pack_g[C, xtile(18) x row(8) x col(24)] whose per-tile 192-element runs
    are contiguous, so each position tile needs only TWO 192-col matmuls
    (lower / upper half of its 16-row window). Each tgt row is packed once
    per group but used by two strips.
  - PSUM tiles hold two position tiles (2 banks, 384-col blocks at 512-col
    offsets); DVE/ACT alternate evacuating pairs (x 1/C, cast bf16),
    halving per-instruction overhead. Raw Gram blocks are DMA'd to DRAM
    densely, spread over all three DMA rings (SWDGE + SP + ACT HWDGE).
  - The host (this file, numpy) de-shears the banded diagonals into the
    [B, 81, H, W] output. The per-position diagonal gather needs a
    partition-coupled byte offset which neither HWDGE nor SWDGE descriptor
    generation supports (verified on HW: HWDGE wraps the sub-row offset
    mod 4 elements, SWDGE emits garbage), so it stays on the host where it
    is a cheap vectorized gather.
  - Inputs are cast to bf16 on the host (halves HBM read traffic); PSUM
    accumulation is fp32. tgt is zero-padded by S=4 on the host so all
    device DMAs are fully contiguous per partition.
"""

import sys

for _p in ("/opt/trn_rl_repo",):
    if _p not in sys.path:
        sys.path.insert(0, _p)

import numpy as np
import ml_dtypes

import concourse.mybir as mybir
import concourse.tile as tile
from concourse import bacc
from concourse.ap import AP
from concourse.bass_utils import run_bass_kernel_spmd

B, C, H, W, S = 8, 128, 160, 288, 4
TY, TX = 8, 16                      # src tile = 8x16 = 128 positions (PSUM partitions)
WIN_Y, WIN_X = TY + 2 * S, TX + 2 * S   # 16 x 24 tgt window
NWIN = WIN_Y * WIN_X                # 384 PSUM columns per tile
NSTRIP = H // TY                    # 20 row strips
NXT = W // TX                       # 18 x tiles per strip
HP, WP = H + 2 * S, W + 2 * S       # padded tgt dims (168, 296)
TGT_CHUNK = 24                      # tgt rows per chunk tile (168 = 7 x 24; never
                                    # splits an 8-row pack group)
N_TGT_CHUNKS = HP // TGT_CHUNK      # 7
N_PACK = HP // TY                   # 21 8-row pack groups
PACK_COLS = NXT * TY * WIN_X        # 3456 packed elements per partition
HALF = TY * WIN_X                   # 192 columns per half-window matmul
PSUM_PAIR = 1024                    # 2 banks: tile j of pair at cols 512*j
N_CORES = 8

BF16 = mybir.dt.bfloat16
NP_BF16 = ml_dtypes.bfloat16


def _displacements(s):
    d = [(0, 0)]
    for i in range(1, s + 1):
        d += [(-i, 0), (i, 0), (0, -i), (0, i)]
        for j in range(1, s + 1):
            d += [(-i, -j), (i, j), (-i, j), (i, -j)]
    return d


DISPLACEMENTS = _displacements(S)


def _build_bass():
    nc = bacc.Bacc(
        "TRN2",
        target_bir_lowering=False,
        debug=False,
        num_devices=N_CORES,
    )
    # src pre-tiled on host: [C, NSTRIP, NXT, TY*TX] so each tile's lhsT is
    # one contiguous 128-element slice.
    src_t = nc.dram_tensor(
        "src", [C, NSTRIP, NXT, TY * TX], BF16, kind="ExternalInput"
    ).ap()
    tgt_t = nc.dram_tensor("tgtp", [C, HP, WP], BF16, kind="ExternalInput").ap()
    out_t = nc.dram_tensor(
        "gram", [NSTRIP, C, NXT, NWIN], BF16, kind="ExternalOutput"
    ).ap()

    with tile.TileContext(nc) as tc:
        with (
            tc.tile_pool(name="tgtres", bufs=1) as tgt_pool,
            tc.tile_pool(name="pack", bufs=4) as pack_pool,
            tc.tile_pool(name="srcstrip", bufs=5) as src_pool,
            tc.tile_pool(name="outstrip", bufs=3) as out_pool,
            tc.tile_pool(name="psum", bufs=4, space="PSUM") as psum_pool,
        ):
            # tgt resident in SBUF, loaded up front in row chunks (separate
            # tiles) so packs only depend on their chunk. The loads ride
            # SWDGE (gpsimd, idle at the head) so their DMA-issue
            # instructions don't sit in front of the pack copies in ACT's
            # strict-FIFO queue. Rescheduling the preload further
            # (just-in-time chunk loads, delaying a ring's write share)
            # measured slower; remaining variants are within the device's
            # ~4% run-to-run noise.
            tgt_chunks = []
            for ci in range(N_TGT_CHUNKS):
                ch = tgt_pool.tile([C, TGT_CHUNK * WP], BF16, tag=f"tgtc{ci}")
                nc.gpsimd.dma_start(
                    ch[:], tgt_t[:, ci * TGT_CHUNK : (ci + 1) * TGT_CHUNK, :]
                )
                tgt_chunks.append(ch)

            packs = {}

            def ensure_pack(g):
                # pack_g[p, t*192 + row*24 + col] = tgt[p, 8g + row, 16t + col]
                if g not in packs:
                    r0 = TY * g
                    ch = tgt_chunks[r0 // TGT_CHUNK]
                    cap = ch[:]
                    src_ap = AP(
                        cap.tensor,
                        cap.offset + (r0 % TGT_CHUNK) * WP,
                        [[cap.shape[1], C], [TX, NXT], [WP, TY], [1, WIN_X]],
                    )
                    pk = pack_pool.tile([C, PACK_COLS], BF16)
                    dst = pk[:].rearrange("p (t r c) -> p t r c", r=TY, c=WIN_X)
                    # DVE/ACT alternate the pack copies (GPSIMD's Q7 cores
                    # are ~4x slower per element and serialize with SWDGE
                    # descriptor generation - measured 267us busy).
                    if g % 2 == 0:
                        nc.vector.tensor_scalar_mul(dst, src_ap, 1.0)
                    else:
                        nc.scalar.mul(dst, src_ap, 1.0)
                    packs[g] = pk
                return packs[g]

            for s in range(NSTRIP):
                src_tile = src_pool.tile([C, NXT * TY * TX], BF16)
                nc.sync.dma_start(src_tile[:], src_t[:, s])
                src_view = src_tile.rearrange("p (t m) -> p t m", m=TY * TX)

                pk_lo = ensure_pack(s)
                pk_hi = ensure_pack(s + 1)

                out_tile = out_pool.tile([C, NXT * NWIN], BF16)
                out_view = out_tile.rearrange("p (t w) -> p t w", w=NWIN)

                for pi in range(NXT // 2):
                    t0 = 2 * pi
                    ps = psum_pool.tile([C, PSUM_PAIR], mybir.dt.float32)
                    for j in (0, 1):
                        t = t0 + j
                        base = 512 * j
                        nc.tensor.matmul(
                            ps[:, base : base + HALF],
                            lhsT=src_view[:, t, :],
                            rhs=pk_lo[:, t * HALF : (t + 1) * HALF],
                            start=True,
                            stop=True,
                        )
                        nc.tensor.matmul(
                            ps[:, base + HALF : base + NWIN],
                            lhsT=src_view[:, t, :],
                            rhs=pk_hi[:, t * HALF : (t + 1) * HALF],
                            start=True,
                            stop=True,
                        )
                    # Evacuate both tiles of the pair in one instruction.
                    pv = ps.rearrange("p (j c) -> p j c", c=512)[:, :, :NWIN]
                    dst = out_view[:, t0 : t0 + 2, :]
                    if pi % 2 == 0:
                        nc.vector.tensor_scalar_mul(dst, pv, 1.0 / C)
                    else:
                        nc.scalar.mul(dst, pv, 1.0 / C)
                    if s == NSTRIP - 1:
                        # Last strip: write each pair as soon as it is
                        # evacuated (HWDGE rings) so the final drain is one
                        # pair, not a whole third of the strip. Its last 4
                        # window rows (cols 288:) lie in the zero padding -
                        # skip them (the output buffer arrives zeroed).
                        eng = nc.sync if pi % 2 == 0 else nc.scalar
                        eng.dma_start(
                            out_t[s, :, t0 : t0 + 2, : 12 * WIN_X],
                            out_view[:, t0 : t0 + 2, : 12 * WIN_X],
                        )

                if s == 0:
                    # Strip 0's first 4 window rows (cols :96) lie in the
                    # zero padding - skip writing them (the output buffer
                    # arrives zeroed), saving 0.44MB of the critical write
                    # stream.
                    nt3 = NXT // 3
                    for bi, eng in ((0, nc.gpsimd), (1, nc.sync), (2, nc.scalar)):
                        nc_lo, nc_hi = bi * nt3, (bi + 1) * nt3
                        eng.dma_start(
                            out_t[s, :, nc_lo:nc_hi, 4 * WIN_X :],
                            out_view[:, nc_lo:nc_hi, 4 * WIN_X :],
                        )
                elif s < NSTRIP - 1:
                    # Spread the gram writes (the dominant HBM traffic)
                    # across all three DMA rings so no single descriptor
                    # queue serializes them.
                    third = (NXT // 3) * NWIN
                    nc.gpsimd.dma_start(
                        out_t[s, :, : NXT // 3], out_tile[:, :third]
                    )
                    nc.sync.dma_start(
                        out_t[s, :, NXT // 3 : 2 * (NXT // 3)],
                        out_tile[:, third : 2 * third],
                    )
                    nc.scalar.dma_start(
                        out_t[s, :, 2 * (NXT // 3) :], out_tile[:, 2 * third :]
                    )

    nc.compile()
    return nc


_NC = None


def _get_nc():
    global _NC
    if _NC is None:
        _NC = _build_bass()
    return _NC


def _run_device(src_bf, tgtp_bf, **run_kwargs):
    nc = _get_nc()
    in_maps = [{"src": src_bf[b], "tgtp": tgtp_bf[b]} for b in range(B)]
    return run_bass_kernel_spmd(nc, in_maps, core_ids=list(range(N_CORES)), **run_kwargs)


def _deshear(gram):
    """gram: [B, NSTRIP, C, NXT, NWIN] (any float dtype) -> [B, 81, H, W] fp32."""
    g = np.asarray(gram, dtype=np.float32).reshape(
        B, NSTRIP, TY, TX, NXT, WIN_Y, WIN_X
    )
    out = np.empty((B, len(DISPLACEMENTS), H, W), np.float32)
    yy = np.arange(TY)[:, None]
    xx = np.arange(TX)[None, :]
    for k, (dy, dx) in enumerate(DISPLACEMENTS):
        # v axes: (yy, xx, b, strip, xtile)
        v = g[:, :, yy, xx, :, yy + dy + S, xx + dx + S]
        out[:, k] = v.transpose(2, 3, 0, 4, 1).reshape(B, H, W)
    return out


def kernel(src, tgt, _profile_out=None):
    src = np.asarray(src)
    tgt = np.asarray(tgt)
    assert src.shape == (B, C, H, W) and tgt.shape == (B, C, H, W)

    # [B, C, H, W] -> [B, C, NSTRIP, TY, NXT, TX] -> [B, C, NSTRIP, NXT, TY*TX]
    src_bf = np.ascontiguousarray(
        src.astype(NP_BF16)
        .reshape(B, C, NSTRIP, TY, NXT, TX)
        .transpose(0, 1, 2, 4, 3, 5)
        .reshape(B, C, NSTRIP, NXT, TY * TX)
    )
    tgtp_bf = np.zeros((B, C, HP, WP), NP_BF16)
    tgtp_bf[:, :, S : S + H, S : S + W] = tgt.astype(NP_BF16)

    kw = {}
    if _profile_out is not None:
        kw["trace"] = True
        import os, shutil

        td = "/tmp/bass_trace"
        shutil.rmtree(td, ignore_errors=True)
        os.makedirs(td, exist_ok=True)
        kw["tmpdir"] = td
    res = _run_device(src_bf, tgtp_bf, **kw)
    if _profile_out is not None:
        _profile_out.update(
            exec_time_ns=res.exec_time_ns,
            mean_exec_time_ns=res.mean_exec_time_ns,
        )

    gram = np.stack([res.results[b]["gram"] for b in range(B)])
    # Regions the device skips writing (window rows fully inside the zero
    # padding). The output buffer is zero-initialized, but zero them here
    # too so correctness never depends on that.
    gram[:, 0, :, :, : 4 * WIN_X] = 0
    gram[:, NSTRIP - 1, :, :, 12 * WIN_X :] = 0
    return _deshear(gram)


# revision 32
# speedup vs baseline: 1.0454x; 1.0060x over previous
"""CostVolumeLayer Trainium2 kernel.

Problem: src, tgt [B=8, C=128, H=160, W=288] fp32.
out[b, k, y, x] = (1/C) * sum_c src[b,c,y,x] * tgt[b,c,y+dy_k,x+dx_k]
for the 81 displacements (dy,dx) in [-4,4]^2 (torch CostVolume channel order),
with zero padding outside the image.

Strategy (data-parallel over batch, one batch per NeuronCore):
  - For each 8x16 tile of src positions, the PE computes the Gram block
    src_tile[C, 128].T @ tgt_window[C, 16x24] -> PSUM [128 pos, 384 window].
    Every (pos, k) output is some element of this block (banded diagonals).
    8x16 tiles minimize window area per position (384 cols vs 480 for 4x32)
    - the Gram write to HBM is the kernel's bottleneck (35.4 MB/core).
  - A naive 16-matmul-per-tile schedule (one 24-col matmul per window row)
    is PE-bound on weight reloads. Instead the window rows are pre-packed:
    for every 8-row group of padded tgt, a DVE/ACT copy builds
    pack_g[C, xtile(18) x row(8) x col(24)] whose per-tile 192-element runs
    are contiguous, so each position tile needs only TWO 192-col matmuls
    (lower / upper half of its 16-row window). Each tgt row is packed once
    per group but used by two strips.
  - PSUM tiles hold two position tiles (2 banks, 384-col blocks at 512-col
    offsets); DVE/ACT alternate evacuating pairs (x 1/C, cast bf16),
    halving per-instruction overhead. Raw Gram blocks are DMA'd to DRAM
    densely, spread over all three DMA rings (SWDGE + SP + ACT HWDGE).
  - The host (this file, numpy) de-shears the banded diagonals into the
    [B, 81, H, W] output. The per-position diagonal gather needs a
    partition-coupled byte offset which neither HWDGE nor SWDGE descriptor
    generation supports (verified on HW: HWDGE wraps the sub-row offset
    mod 4 elements, SWDGE emits garbage), so it stays on the host where it
    is a cheap vectorized gather.
  - Inputs are cast to bf16 on the host (halves HBM read traffic); PSUM
    accumulation is fp32. tgt is zero-padded by S=4 on the host so all
    device DMAs are fully contiguous per partition.
"""

import sys

for _p in ("/opt/trn_rl_repo",):
    if _p not in sys.path:
        sys.path.insert(0, _p)

import numpy as np
import ml_dtypes

import concourse.mybir as mybir
import concourse.tile as tile
from concourse import bacc
from concourse.ap import AP
from concourse.bass_utils import run_bass_kernel_spmd

B, C, H, W, S = 8, 128, 160, 288, 4
TY, TX = 8, 16                      # src tile = 8x16 = 128 positions (PSUM partitions)
WIN_Y, WIN_X = TY + 2 * S, TX + 2 * S   # 16 x 24 tgt window
NWIN = WIN_Y * WIN_X                # 384 PSUM columns per tile
NSTRIP = H // TY                    # 20 row strips
NXT = W // TX                       # 18 x tiles per strip
HP, WP = H + 2 * S, W + 2 * S       # padded tgt dims (168, 296)
TGT_CHUNK = 24                      # tgt rows per chunk tile (168 = 7 x 24; never
                                    # splits an 8-row pack group)
N_TGT_CHUNKS = HP // TGT_CHUNK      # 7
N_PACK = HP // TY                   # 21 8-row pack groups
PACK_COLS = NXT * TY * WIN_X        # 3456 packed elements per partition
HALF = TY * WIN_X                   # 192 columns per half-window matmul
PSUM_PAIR = 1024                    # 2 banks: tile j of pair at cols 512*j
N_CORES = 8

BF16 = mybir.dt.bfloat16
NP_BF16 = ml_dtypes.bfloat16


def _displacements(s):
    d = [(0, 0)]
    for i in range(1, s + 1):
        d += [(-i, 0), (i, 0), (0, -i), (0, i)]
        for j in range(1, s + 1):
            d += [(-i, -j), (i, j), (-i, j), (i, -j)]
    return d


DISPLACEMENTS = _displacements(S)


def _build_bass():
    nc = bacc.Bacc(
        "TRN2",
        target_bir_lowering=False,
        debug=False,
        num_devices=N_CORES,
    )
    # src pre-tiled on host: [C, NSTRIP, NXT, TY*TX] so each tile's lhsT is
    # one contiguous 128-element slice.
    src_t = nc.dram_tensor(
        "src", [C, NSTRIP, NXT, TY * TX], BF16, kind="ExternalInput"
    ).ap()
    tgt_t = nc.dram_tensor("tgtp", [C, HP, WP], BF16, kind="ExternalInput").ap()
    out_t = nc.dram_tensor(
        "gram", [NSTRIP, C, NXT, NWIN], BF16, kind="ExternalOutput"
    ).ap()

    with tile.TileContext(nc) as tc:
        with (
            tc.tile_pool(name="tgtres", bufs=1) as tgt_pool,
            tc.tile_pool(name="pack", bufs=4) as pack_pool,
            tc.tile_pool(name="srcstrip", bufs=4) as src_pool,
            tc.tile_pool(name="outstrip", bufs=4) as out_pool,
            tc.tile_pool(name="psum", bufs=4, space="PSUM") as psum_pool,
        ):
            # tgt resident in SBUF, loaded up front in row chunks (separate
            # tiles) so packs only depend on their chunk. The loads ride
            # SWDGE (gpsimd, idle at the head) so their DMA-issue
            # instructions don't sit in front of the pack copies in ACT's
            # strict-FIFO queue. Rescheduling the preload further
            # (just-in-time chunk loads, delaying a ring's write share)
            # measured slower; remaining variants are within the device's
            # ~4% run-to-run noise.
            tgt_chunks = []
            for ci in range(N_TGT_CHUNKS):
                ch = tgt_pool.tile([C, TGT_CHUNK * WP], BF16, tag=f"tgtc{ci}")
                nc.gpsimd.dma_start(
                    ch[:], tgt_t[:, ci * TGT_CHUNK : (ci + 1) * TGT_CHUNK, :]
                )
                tgt_chunks.append(ch)

            packs = {}

            def ensure_pack(g):
                # pack_g[p, t*192 + row*24 + col] = tgt[p, 8g + row, 16t + col]
                if g not in packs:
                    r0 = TY * g
                    ch = tgt_chunks[r0 // TGT_CHUNK]
                    cap = ch[:]
                    src_ap = AP(
                        cap.tensor,
                        cap.offset + (r0 % TGT_CHUNK) * WP,
                        [[cap.shape[1], C], [TX, NXT], [WP, TY], [1, WIN_X]],
                    )
                    pk = pack_pool.tile([C, PACK_COLS], BF16)
                    dst = pk[:].rearrange("p (t r c) -> p t r c", r=TY, c=WIN_X)
                    # DVE/ACT alternate the pack copies (GPSIMD's Q7 cores
                    # are ~4x slower per element and serialize with SWDGE
                    # descriptor generation - measured 267us busy).
                    if g % 2 == 0:
                        nc.vector.tensor_scalar_mul(dst, src_ap, 1.0)
                    else:
                        nc.scalar.mul(dst, src_ap, 1.0)
                    packs[g] = pk
                return packs[g]

            for s in range(NSTRIP):
                src_tile = src_pool.tile([C, NXT * TY * TX], BF16)
                nc.sync.dma_start(src_tile[:], src_t[:, s])
                src_view = src_tile.rearrange("p (t m) -> p t m", m=TY * TX)

                pk_lo = ensure_pack(s)
                pk_hi = ensure_pack(s + 1)

                out_tile = out_pool.tile([C, NXT * NWIN], BF16)
                out_view = out_tile.rearrange("p (t w) -> p t w", w=NWIN)

                for pi in range(NXT // 2):
                    t0 = 2 * pi
                    ps = psum_pool.tile([C, PSUM_PAIR], mybir.dt.float32)
                    for j in (0, 1):
                        t = t0 + j
                        base = 512 * j
                        nc.tensor.matmul(
                            ps[:, base : base + HALF],
                            lhsT=src_view[:, t, :],
                            rhs=pk_lo[:, t * HALF : (t + 1) * HALF],
                            start=True,
                            stop=True,
                        )
                        nc.tensor.matmul(
                            ps[:, base + HALF : base + NWIN],
                            lhsT=src_view[:, t, :],
                            rhs=pk_hi[:, t * HALF : (t + 1) * HALF],
                            start=True,
                            stop=True,
                        )
                    # Evacuate both tiles of the pair in one instruction.
                    pv = ps.rearrange("p (j c) -> p j c", c=512)[:, :, :NWIN]
                    dst = out_view[:, t0 : t0 + 2, :]
                    if pi % 2 == 0:
                        nc.vector.tensor_scalar_mul(dst, pv, 1.0 / C)
                    else:
                        nc.scalar.mul(dst, pv, 1.0 / C)
                    if s == NSTRIP - 1:
                        # Last strip: write each pair as soon as it is
                        # evacuated (HWDGE rings) so the final drain is one
                        # pair, not a whole third of the strip. Its last 4
                        # window rows (cols 288:) lie in the zero padding -
                        # skip them (the output buffer arrives zeroed).
                        eng = nc.sync if pi % 2 == 0 else nc.scalar
                        eng.dma_start(
                            out_t[s, :, t0 : t0 + 2, : 12 * WIN_X],
                            out_view[:, t0 : t0 + 2, : 12 * WIN_X],
                        )

                if s == 0:
                    # Strip 0's first 4 window rows (cols :96) lie in the
                    # zero padding - skip writing them (the output buffer
                    # arrives zeroed), saving 0.44MB of the critical write
                    # stream.
                    nt3 = NXT // 3
                    for bi, eng in ((0, nc.gpsimd), (1, nc.sync), (2, nc.scalar)):
                        nc_lo, nc_hi = bi * nt3, (bi + 1) * nt3
                        eng.dma_start(
                            out_t[s, :, nc_lo:nc_hi, 4 * WIN_X :],
                            out_view[:, nc_lo:nc_hi, 4 * WIN_X :],
                        )
                elif s < NSTRIP - 1:
                    # Spread the gram writes (the dominant HBM traffic)
                    # across all three DMA rings so no single descriptor
                    # queue serializes them.
                    third = (NXT // 3) * NWIN
                    nc.gpsimd.dma_start(
                        out_t[s, :, : NXT // 3], out_tile[:, :third]
                    )
                    nc.sync.dma_start(
                        out_t[s, :, NXT // 3 : 2 * (NXT // 3)],
                        out_tile[:, third : 2 * third],
                    )
                    nc.scalar.dma_start(
                        out_t[s, :, 2 * (NXT // 3) :], out_tile[:, 2 * third :]
                    )

    nc.compile()
    return nc


_NC = None


def _get_nc():
    global _NC
    if _NC is None:
        _NC = _build_bass()
    return _NC


def _run_device(src_bf, tgtp_bf, **run_kwargs):
    nc = _get_nc()
    in_maps = [{"src": src_bf[b], "tgtp": tgtp_bf[b]} for b in range(B)]
    return run_bass_kernel_spmd(nc, in_maps, core_ids=list(range(N_CORES)), **run_kwargs)


def _deshear(gram):
    """gram: [B, NSTRIP, C, NXT, NWIN] (any float dtype) -> [B, 81, H, W] fp32."""
    g = np.asarray(gram, dtype=np.float32).reshape(
        B, NSTRIP, TY, TX, NXT, WIN_Y, WIN_X
    )
    out = np.empty((B, len(DISPLACEMENTS), H, W), np.float32)
    yy = np.arange(TY)[:, None]
    xx = np.arange(TX)[None, :]
    for k, (dy, dx) in enumerate(DISPLACEMENTS):
        # v axes: (yy, xx, b, strip, xtile)
        v = g[:, :, yy, xx, :, yy + dy + S, xx + dx + S]
        out[:, k] = v.transpose(2, 3, 0, 4, 1).reshape(B, H, W)
    return out


def kernel(src, tgt, _profile_out=None):
    src = np.asarray(src)
    tgt = np.asarray(tgt)
    assert src.shape == (B, C, H, W) and tgt.shape == (B, C, H, W)

    # [B, C, H, W] -> [B, C, NSTRIP, TY, NXT, TX] -> [B, C, NSTRIP, NXT, TY*TX]
    src_bf = np.ascontiguousarray(
        src.astype(NP_BF16)
        .reshape(B, C, NSTRIP, TY, NXT, TX)
        .transpose(0, 1, 2, 4, 3, 5)
        .reshape(B, C, NSTRIP, NXT, TY * TX)
    )
    tgtp_bf = np.zeros((B, C, HP, WP), NP_BF16)
    tgtp_bf[:, :, S : S + H, S : S + W] = tgt.astype(NP_BF16)

    kw = {}
    if _profile_out is not None:
        kw["trace"] = True
        import os, shutil

        td = "/tmp/bass_trace"
        shutil.rmtree(td, ignore_errors=True)
        os.makedirs(td, exist_ok=True)
        kw["tmpdir"] = td
    res = _run_device(src_bf, tgtp_bf, **kw)
    if _profile_out is not None:
        _profile_out.update(
            exec_time_ns=res.exec_time_ns,
            mean_exec_time_ns=res.mean_exec_time_ns,
        )

    gram = np.stack([res.results[b]["gram"] for b in range(B)])
    # Regions the device skips writing (window rows fully inside the zero
    # padding). The output buffer is zero-initialized, but zero them here
    # too so correctness never depends on that.
    gram[:, 0, :, :, : 4 * WIN_X] = 0
    gram[:, NSTRIP - 1, :, :, 12 * WIN_X :] = 0
    return _deshear(gram)
